# revision 23
# baseline (speedup 1.0000x reference)
"""Trainium2 Bass kernel for nn_BlankEmbedding (embedding gather + blank-run scan).

Math: the reference computes e = emb_table[x], then runs 8 iterations of
    pos = shift_right(pos); acc = shift_right(acc); out = out + acc; acc = out*pos
starting from pos = is_preblank.  Unrolling, out[i] = sum_d C[i,d] * e[i-d]
with banded integer coefficients C that depend only on x.  Rows with any
C[i,d>0] != 0 are rare (~1/16, grouped <=128 per output half).

The whole kernel runs in bf16 (the harness gate is rel_err < 2e-2; bf16
round-off lands ~8e-3), which halves every DMA byte moved.

  per core (2048 of the 16384 rows, data-parallel over B*S):
    1. dma_gather the core's embedding rows from a deduplicated bf16 table
       (HBM->SBUF, chunks [512,512,512,384,128], each chunk with its own
       SBUF buffer so the gather stream never stalls), writing each chunk
       out with a strided DMA on one HWDGE ring.
    2. affected rows grouped <=128 by output half (sorted by band length):
       per-depth dma_gathers on SWDGE queue 1 (so band bytes never queue
       behind the main stream) fetch e[i-d] into band slots that DVE
       pre-zeroed (DVE is otherwise idle); dead in-count slots index a zero
       row appended to the table.  The delta for each group is built on the
       TENSOR engine as sum_j diag(coef_j) @ band_j accumulated in PSUM
       (f32), the scalar engine copies PSUM -> SBUF bf16, and a
       dma_scatter_add on queue 2 applies it onto the already-written
       output rows (the reg-loaded count skips partitions past the group's
       row count).

Host side only computes index lists / coefficient diagonals from x ([B,S]
int ops) and reassembles the 8 per-core outputs (upcasting bf16 -> f32).
"""

import numpy as np

B, S, D = 4, 4096, 2048
N_CORES = 8
RPC = (B * S) // N_CORES          # rows per core = 2048
CHUNK_SIZES = [512, 512, 512, 384, 128]
N_CHUNKS = len(CHUNK_SIZES)
CHUNK_OFF = [sum(CHUNK_SIZES[:i]) for i in range(N_CHUNKS + 1)]
GPPS = [cs // 128 for cs in CHUNK_SIZES]  # rows per partition per chunk
CPCS = [cs // 16 for cs in CHUNK_SIZES]   # idx columns per chunk
CPC_OFF = [sum(CPCS[:i]) for i in range(N_CHUNKS + 1)]
NBLANK_IDS = 16
N_ITER = 8
BAND = N_ITER + 1                 # out[i] depends on e[i-8..i]
PSW = 512                         # psum bank width (f32)
NPC = D // PSW                    # psum column chunks per row


def _cdiv(a, b):
    return (a + b - 1) // b


def _compute_coeffs(x):
    """C[b, s, d] for d=0..8 (float64 holds small ints exactly)."""
    b, s = x.shape
    blank = ((x >= 0) & (x < NBLANK_IDS)).astype(np.float64)
    shift_r = lambda t: np.concatenate([np.zeros_like(t[:, :1]), t[:, :-1]], axis=1)
    first = np.maximum(blank - shift_r(blank), 0.0)
    m = np.concatenate([first[:, 1:], np.zeros_like(first[:, :1])], axis=1)
    C = np.zeros((b, s, BAND))
    C[:, :, 0] = 1.0
    for k in range(1, N_ITER + 1):
        m_k = np.zeros_like(m)
        m_k[:, k:] = m[:, :-k]
        Cs = np.zeros_like(C)
        Cs[:, 1:, 1:] = C[:, :-1, :-1]
        C = C + m_k[:, :, None] * Cs
    return C


def _wrap16(vals, ncols):
    """Wrap a 1-D index list into the [128, ncols] int16 layout the SWDGE
    gather/scatter ucode expects: slot j at [j % 16, j // 16], replicated
    across the eight 16-partition Q7 core groups."""
    blk = np.zeros((16, ncols), dtype=np.int16)
    v = np.asarray(vals, dtype=np.int16)
    for j in range(len(v)):
        blk[j % 16, j // 16] = v[j]
    return np.tile(blk, (8, 1))


def _prepare(x_np):
    """All host-side index/coefficient prep. Returns per-core arrays + meta.

    The device table is emb[uniq] with one extra all-zero row at index NV:
    dead in-count band-gather slots index it so they contribute exact zeros.
    """
    uniq, inv = np.unique(x_np, return_inverse=True)
    ridx = inv.reshape(x_np.shape).astype(np.int64)
    NV = len(uniq)
    assert NV + 1 <= 32767, "int16 gather index overflow"
    ZROW = NV                                         # the appended zero row

    C = _compute_coeffs(x_np)
    aff = (C[:, :, 1:] != 0).any(axis=2)              # [B,S]

    cores = []
    for c in range(N_CORES):
        b, h = c // 2, c % 2
        s0 = h * RPC
        midx = np.zeros((128, CPC_OFF[-1]), dtype=np.int16)
        for ch in range(N_CHUNKS):
            cs, gpp = CHUNK_SIZES[ch], GPPS[ch]
            slots = np.empty(cs, dtype=np.int64)
            for j in range(cs):
                l = (j % 128) * gpp + (j // 128) + CHUNK_OFF[ch]
                slots[j] = ridx[b, s0 + l]
            midx[:, CPC_OFF[ch]:CPC_OFF[ch + 1]] = _wrap16(slots, CPCS[ch])

        rows_all = np.nonzero(aff[b, s0:s0 + RPC])[0]
        Cc = C[b, s0:s0 + RPC]                        # [RPC, 9]
        halves = []
        for hh in range(2):
            rh = rows_all[(rows_all >= hh * (RPC // 2))
                          & (rows_all < (hh + 1) * (RPC // 2))]
            if len(rh):
                blen = np.array([np.nonzero(Cc[r, 1:])[0].max() + 1 for r in rh])
                rh = rh[np.argsort(-blen, kind="stable")]
            halves.append(rh)
        cores.append(dict(b=b, s0=s0, halves=halves, Cc=Cc, midx=midx))

    # group g of half h waits for the writebacks covering that half
    H = [max(_cdiv(len(co["halves"][h]), 128) for co in cores) for h in range(2)]
    G = H[0] + H[1]
    meta = dict(NV=NV, G=G, Ls=[], wait_chunks=[])
    if G == 0:
        for co in cores:
            co.update(bidx=None, sidx=None, cdiag=None, cnts=None)
        return uniq, cores, meta
    group_defs = []   # (half, start_within_half)
    for h in range(2):
        for k in range(H[h]):
            group_defs.append((h, k * 128))
            meta["wait_chunks"].append(2 if h == 0 else N_CHUNKS)
    for co in cores:
        co["rows_g"] = [co["halves"][h][st:st + 128] for h, st in group_defs]

    # active depth list per group = union over cores; slot j holds depth d_j
    depth_sets = [set() for _ in range(G)]
    for co in cores:
        Cc = co["Cc"]
        for g in range(G):
            rg = co["rows_g"][g]
            for d in range(1, N_ITER + 1):
                if len(rg) and (Cc[rg, d] != 0).any():
                    depth_sets[g].add(d)
    depths = [sorted(ds) if ds else [1] for ds in depth_sets]
    Ls = [len(ds) for ds in depths]
    meta["Ls"] = Ls
    meta["depths"] = depths
    LT = sum(Ls)
    SLOT_OFF = [sum(Ls[:g]) for g in range(G + 1)]

    for co in cores:
        b, s0, Cc = co["b"], co["s0"], co["Cc"]
        bidx = np.zeros((128, LT * 8), dtype=np.int16)
        sidx = np.zeros((128, G * 8), dtype=np.int16)
        cdiag = np.zeros((128, LT * 128), dtype=np.float32)
        # cnts[0, LT + g] = scatter count; cnts[0, slot] = gather count for
        # that slot (rows are length-sorted so rows needing depth d form a
        # prefix; partitions past the count stay at the DVE-memset zeros)
        cnts = np.zeros((1, LT + G), dtype=np.int32)
        for g in range(G):
            rg = co["rows_g"][g]
            n_rows = max(len(rg), 1)
            cnts[0, LT + g] = n_rows
            for j, d in enumerate(depths[g]):
                slot = SLOT_OFF[g] + j
                vals = np.full(128, -1, dtype=np.int64)
                n_gd = 1
                for p in range(n_rows):
                    if p < len(rg) and Cc[rg[p], d] != 0:
                        vals[p] = ridx[b, s0 + int(rg[p]) - d]
                        cdiag[p, slot * 128 + p] = Cc[rg[p], d]
                        n_gd = p + 1
                    else:
                        vals[p] = ZROW                # exact-zero pad row
                vals[n_gd:] = -1                      # trailing: skipped
                cnts[0, slot] = n_gd
                bidx[:, slot * 8:slot * 8 + 8] = _wrap16(vals, 8)

            tgts = np.full(128, -1, dtype=np.int64)
            if len(rg):
                tgts[:len(rg)] = rg
            else:
                tgts[0] = 0   # adds an exact 0 to row 0
            sidx[:, g * 8:(g + 1) * 8] = _wrap16(tgts, 8)
        co.update(bidx=bidx, sidx=sidx, cdiag=cdiag, cnts=cnts)
    return uniq, cores, meta


def _build_program(NV, G, Ls, wait_chunks):
    import concourse.bacc as bacc
    import concourse.mybir as mybir
    from concourse.library_config import mlp

    f32, i16, i32 = mybir.dt.float32, mybir.dt.int16, mybir.dt.int32
    bf16 = mybir.dt.bfloat16
    COPY = mybir.ActivationFunctionType.Copy

    nc = bacc.Bacc("TRN2", target_bir_lowering=False, debug=False,
                   enable_asserts=False, num_devices=N_CORES,
                   num_swdge_queues=3, dynamic_dma_scratch_size=65536)
    table = nc.dram_tensor("table", [NV + 1, D], bf16, kind="ExternalInput")
    midx_d = nc.dram_tensor("midx", [128, CPC_OFF[-1]], i16, kind="ExternalInput")
    out_d = nc.dram_tensor("out", [RPC, D], bf16, kind="ExternalOutput")
    LT = sum(Ls)
    SLOT_OFF = [sum(Ls[:g]) for g in range(G + 1)]
    if G:
        bidx_d = nc.dram_tensor("bidx", [128, LT * 8], i16, kind="ExternalInput")
        sidx_d = nc.dram_tensor("sidx", [128, G * 8], i16, kind="ExternalInput")
        cdiag_d = nc.dram_tensor("cdiag", [128, LT * 128], bf16,
                                 kind="ExternalInput")
        cnts_d = nc.dram_tensor("cnts", [1, LT + G], i32, kind="ExternalInput")

    from contextlib import ExitStack
    with ExitStack() as st:
        # every chunk gets its own buffer (bf16 halves SBUF): no reuse waits
        mbuf = [st.enter_context(nc.sbuf_tensor(f"mbuf{i}", [128, GPPS[i], D], bf16))
                for i in range(N_CHUNKS)]
        midx_s = st.enter_context(nc.sbuf_tensor("midx_s", [128, CPC_OFF[-1]], i16))
        m_sem = st.enter_context(nc.semaphore("m_sem"))
        g_sems = [st.enter_context(nc.semaphore(f"g_sem{c}")) for c in range(N_CHUNKS)]
        w_sems = [st.enter_context(nc.semaphore(f"w_sem{c}")) for c in range(N_CHUNKS)]
        if G:
            bands = [st.enter_context(
                nc.sbuf_tensor(f"band{g}", [128, Ls[g], D], bf16))
                for g in range(G)]
            deltas = [st.enter_context(
                nc.sbuf_tensor(f"delta{g}", [128, 1, D], bf16))
                for g in range(G)]
            npsets = min(G, 2)
            psum = [[st.enter_context(
                nc.psum_tensor(f"ps{e}_{c}", [128, PSW], f32))
                for c in range(NPC)] for e in range(npsets)]
            bidx_s = st.enter_context(nc.sbuf_tensor("bidx_s", [128, LT * 8], i16))
            sidx_s = st.enter_context(nc.sbuf_tensor("sidx_s", [128, G * 8], i16))
            cdiag_s = st.enter_context(
                nc.sbuf_tensor("cdiag_s", [128, LT * 128], bf16))
            cnts_s = st.enter_context(nc.sbuf_tensor("cnts_s", [1, LT + G], i32))
            nregs = [st.enter_context(nc.gpsimd.register(f"nreg{g}"))
                     for g in range(G)]
            sreg = st.enter_context(nc.gpsimd.register("sreg"))
            bi_sem = st.enter_context(nc.semaphore("bi_sem"))
            si_sem = st.enter_context(nc.semaphore("si_sem"))
            cd_sem = st.enter_context(nc.semaphore("cd_sem"))
            ms_sem = st.enter_context(nc.semaphore("ms_sem"))
            # one sem per band slot: drains on queue 1 overlap, so a shared
            # per-group count would not imply slot j actually landed
            b_sems = [[st.enter_context(nc.semaphore(f"b_sem{g}_{j}"))
                       for j in range(Ls[g])] for g in range(G)]
            pe_sems = [st.enter_context(nc.semaphore(f"pe_sem{g}")) for g in range(G)]
            d_sems = [st.enter_context(nc.semaphore(f"d_sem{g}")) for g in range(G)]
            s_sem = st.enter_context(nc.semaphore("s_sem"))
        block = st.enter_context(nc.Block())

        def writeback(eng, ch):
            eng.wait_ge(g_sems[ch], 16)
            dst = out_d[CHUNK_OFF[ch]:CHUNK_OFF[ch + 1], :].rearrange(
                "(p g) e -> p g e", g=GPPS[ch])
            eng.dma_start(dst, mbuf[ch][:, :, :]).then_inc(w_sems[ch], 16)

        @block.sync
        def _(sync):
            sync.dma_start(midx_s[:, :], midx_d[:, :]).then_inc(m_sem, 16)
            for ch in range(N_CHUNKS):
                writeback(sync, ch)

        @block.scalar
        def _(scalar):
            if G:
                scalar.dma_start(bidx_s[:, :], bidx_d[:, :]).then_inc(bi_sem, 16)
                scalar.dma_start(cnts_s[:, :], cnts_d[:, :]).then_inc(bi_sem, 16)
                scalar.dma_start(cdiag_s[:, :], cdiag_d[:, :]).then_inc(cd_sem, 16)
                scalar.dma_start(sidx_s[:, :], sidx_d[:, :]).then_inc(si_sem, 16)

        @block.gpsimd
        def _(gp):
            gp.load_library(mlp)
            gp.wait_ge(m_sem, 16)

            def main_gather(ch):
                cs = CHUNK_SIZES[ch]
                gp.dma_gather(mbuf[ch][:, :, :], table[:, :],
                              midx_s[:, CPC_OFF[ch]:CPC_OFF[ch + 1]],
                              cs, cs, D,
                              single_packet=False).then_inc(g_sems[ch], 16)

            def band_gathers(g):
                gp.reg_load(nregs[g], cnts_s[0:1, LT + g:LT + g + 1])
                for j in range(Ls[g]):
                    slot = SLOT_OFF[g] + j
                    gp.wait_ge(ms_sem, slot + 1)
                    gp.reg_load(sreg, cnts_s[0:1, slot:slot + 1])
                    gp.dma_gather(bands[g][:, j:j + 1, :], table[:, :],
                                  bidx_s[:, slot * 8:slot * 8 + 8],
                                  128, sreg, D,
                                  single_packet=False,
                                  queue_num=1).then_inc(b_sems[g][j], 16)

            main_gather(0)
            main_gather(1)
            if G:
                gp.wait_ge(bi_sem, 32)
                for g in range(G):
                    band_gathers(g)
            for ch in range(2, N_CHUNKS):
                main_gather(ch)
            if G:
                gp.wait_ge(si_sem, 16)
                for g in range(G):
                    for c in range(wait_chunks[g]):
                        gp.wait_ge(w_sems[c], 16)
                    gp.wait_ge(d_sems[g], 1)
                    gp.dma_scatter_add(out_d[:, :], deltas[g][:, :, :],
                                       sidx_s[:, g * 8:(g + 1) * 8],
                                       128, nregs[g], D,
                                       single_packet=False,
                                       queue_num=2).then_inc(s_sem, 16)
                gp.wait_ge(s_sem, 16 * G)

        @block.tensor
        def _(pe):
            if not G:
                return
            pe.wait_ge(cd_sem, 16)
            for g in range(G):
                e = g % 2
                L = Ls[g]
                if g >= 2:
                    pe.wait_ge(d_sems[g - 2], 1)   # psum set free again
                for j in range(L):
                    slot = SLOT_OFF[g] + j
                    pe.wait_ge(b_sems[g][j], 16)
                    for c in range(NPC):
                        ins = pe.matmul(
                            psum[e][c][:, :],
                            cdiag_s[:, slot * 128:(slot + 1) * 128],
                            bands[g][:, j, c * PSW:(c + 1) * PSW],
                            start=(j == 0), stop=(j == L - 1))
                ins.then_inc(pe_sems[g], 1)

        @block.vector
        def _(v):
            if not G:
                return
            # DVE pre-zeroes band slots (partitions past each slot's gather
            # count must be exact zeros), then copies PSUM deltas to SBUF.
            for g in range(G):
                for j in range(Ls[g]):
                    v.memset(bands[g][:, j:j + 1, :], 0.0).then_inc(ms_sem, 1)
            for g in range(G):
                v.wait_ge(pe_sems[g], 1)
                for c in range(NPC):
                    ins = v.tensor_copy(deltas[g][:, 0, c * PSW:(c + 1) * PSW],
                                        psum[g % 2][c][:, :])
                ins.then_inc(d_sems[g], 1)

    nc.compile()
    return nc


_CACHE = {}
_LAST_RESULT = None


def kernel(x, emb_table):
    global _LAST_RESULT
    import ml_dtypes
    from concourse.bass_utils import run_bass_kernel_spmd

    x_np = np.asarray(x)
    emb_np = np.asarray(emb_table, dtype=np.float32)
    uniq, cores, meta = _prepare(x_np)
    table_sl = np.zeros((meta["NV"] + 1, D), dtype=ml_dtypes.bfloat16)
    table_sl[:meta["NV"]] = emb_np[uniq].astype(ml_dtypes.bfloat16)

    key = (meta["NV"], meta["G"], tuple(meta["Ls"]),
           tuple(meta["wait_chunks"]))
    if key not in _CACHE:
        _CACHE[key] = _build_program(meta["NV"], meta["G"], meta["Ls"],
                                     meta["wait_chunks"])
    nc = _CACHE[key]

    in_maps = []
    for co in cores:
        m = {"table": table_sl, "midx": co["midx"]}
        if meta["G"]:
            m.update(bidx=co["bidx"], sidx=co["sidx"],
                     cdiag=co["cdiag"].astype(ml_dtypes.bfloat16),
                     cnts=co["cnts"])
        in_maps.append(m)

    res = run_bass_kernel_spmd(nc, in_maps, core_ids=list(range(N_CORES)))
    _LAST_RESULT = res
    full = np.empty((B, S, D), dtype=np.float32)
    for c in range(N_CORES):
        b, h = c // 2, c % 2
        full[b, h * RPC:(h + 1) * RPC, :] = res.results[c]["out"].astype(np.float32)
    return full
